# revision 1
# baseline (speedup 1.0000x reference)
"""Trainium2 Bass kernel for nn_CP_Based (CP-decomposition feature-product layer).

Math: out[b,u] = sum_r prod_f ( x0[b,f]*K[0,r,f,u] + x1[b,f]*K[1,r,f,u] )
  with x0 = 1/sqrt(1+X^2), x1 = X/sqrt(1+X^2).
Factor the normalization out of the f-product:
  out[b,u] = S[b] * sum_r prod_f ( K0[f,ru] + X[b,f]*K1[f,ru] ),
  S[b] = 1/sqrt(prod_f (1+X[b,f]^2)).
The 32-feature product is decomposed into 8 groups of 4 features. Each group's
product is a linear map from the 16 multilinear monomials of its 4 features:
  G_g[b,ru] = sum_m Q_g[b,m] * C_g[m,ru]        (K=32 matmul on TensorE)
with C_g packed on the host from `kernel` (tiny; zero rows pad each group to
32 so every matmul slice is 32-partition aligned). Monomials Q are built
batched for 512 rows at a time on VectorE, transposed via TensorE into wide
PSUM tiles so the monomial index lands on the contraction axis, copied once
per macro to SBUF (ScalarE), then 8 matmuls produce G_g and a 7-multiply
elementwise chain forms prod_g G_g; an indicator matmul sums over rank.

Sharding: pure data-parallel over batch: 131072 rows -> 8 cores x 16384.
"""

import sys

import numpy as np

sys.path.insert(0, "/opt/trn_rl_repo")

import concourse.bacc as bacc  # noqa: E402
import concourse.mybir as mybir  # noqa: E402
from concourse.bass_utils import run_bass_kernel_spmd  # noqa: E402
from concourse.tile import TileContext  # noqa: E402

F32 = mybir.dt.float32
AF = mybir.ActivationFunctionType
OP = mybir.AluOpType
AX = mybir.AxisListType

B_FULL = 131072
N_CORES = 8
B_CORE = B_FULL // N_CORES  # 16384
F = 32
R, U = 10, 8
RU = R * U  # 80
NG = 8  # feature groups of 4
TILE_B = 128
CHUNK = 4  # b-subtiles per macro tile -> N=512 matmuls
MACRO_B = TILE_B * CHUNK  # 512
N_MACRO = B_CORE // MACRO_B  # 32
CG = CHUNK * NG  # 32 (chunk, group) pairs


def build_nc():
    nc = bacc.Bacc()
    # host pre-arranges X as [macro, partition, chunk, feature] so each
    # macro's load is one contiguous 64 KB DMA
    X = nc.dram_tensor(
        "X", [N_MACRO, TILE_B, CHUNK, F], F32, kind="ExternalInput"
    )
    C = nc.dram_tensor("C", [128, 2 * RU], F32, kind="ExternalInput")
    ident = nc.dram_tensor("ident", [128, 128], F32, kind="ExternalInput")
    rind = nc.dram_tensor("rind", [RU, U], F32, kind="ExternalInput")
    out = nc.dram_tensor(
        "out", [N_MACRO, U, MACRO_B], F32, kind="ExternalOutput"
    )

    with TileContext(nc) as tc:
        with (
            tc.tile_pool(name="const", bufs=1) as cpool,
            tc.tile_pool(name="xin", bufs=3) as xpool,
            tc.tile_pool(name="work", bufs=3) as wpool,
            tc.tile_pool(name="qts", bufs=4) as qpool,
            tc.tile_pool(name="ps_t", bufs=2, space="PSUM") as tps,
            tc.tile_pool(name="ps_g", bufs=1, space="PSUM") as gps,
            tc.tile_pool(name="ps_o", bufs=2, space="PSUM") as ops_,
        ):
            c_sb = [
                cpool.tile([64, 2 * RU], F32, tag=f"c{h}", name=f"c{h}")
                for h in range(2)
            ]
            id_sb = cpool.tile([128, 128], F32, tag="id")
            ri_sb = cpool.tile([RU, U], F32, tag="ri")
            for h in range(2):
                nc.sync.dma_start(out=c_sb[h][:], in_=C[64 * h : 64 * (h + 1), :])
            nc.sync.dma_start(out=id_sb[:], in_=ident[:, :])
            nc.sync.dma_start(out=ri_sb[:], in_=rind[:, :])

            for mi in range(N_MACRO):
                b0 = mi * MACRO_B
                # x for 4 chunks: [128 b, 4 c, 32 f]
                xm = xpool.tile([TILE_B, CHUNK, F], F32, tag="x")
                nc.gpsimd.dma_start(out=xm[:], in_=X[mi])

                # --- S = 1/sqrt(prod_f (1+x^2)) for all 4 chunks ---
                sq = wpool.tile([TILE_B, CHUNK, F], F32, tag="sq")
                s_p = wpool.tile([TILE_B, CHUNK], F32, tag="s_p")
                s_r = wpool.tile([TILE_B, CHUNK], F32, tag="s_r")
                s_t = wpool.tile([TILE_B, CHUNK], F32, tag="s_t")
                nc.vector.tensor_mul(sq[:], xm[:], xm[:])
                nc.vector.tensor_scalar_add(sq[:], sq[:], 1.0)
                nc.vector.tensor_reduce(s_p[:], sq[:], AX.X, OP.mult)
                nc.vector.reciprocal(s_r[:], s_p[:])
                nc.scalar.sqrt(s_t[:], s_r[:])

                # --- monomial halves, batched over (chunk, group) = cg ---
                # pab[128, cg, 4] = (1, Xa, Xb, XaXb); pcd[128, cg, 4]
                pab = wpool.tile([TILE_B, CG, 4], F32, tag="pab")
                pcd = wpool.tile([TILE_B, CG, 4], F32, tag="pcd")
                xg = xm[:].rearrange("p c (g j) -> p (c g) j", j=4)
                nc.vector.memset(pab[:, :, 0:1], 1.0)
                nc.vector.memset(pcd[:, :, 0:1], 1.0)
                nc.vector.tensor_copy(pab[:, :, 1:3], xg[:, :, 0:2])
                nc.vector.tensor_copy(pcd[:, :, 1:3], xg[:, :, 2:4])
                nc.vector.tensor_mul(pab[:, :, 3:4], xg[:, :, 0:1], xg[:, :, 1:2])
                nc.vector.tensor_mul(pcd[:, :, 3:4], xg[:, :, 2:3], xg[:, :, 3:4])
                # fold S_c into group 0 of each chunk
                for c in range(CHUNK):
                    nc.vector.tensor_scalar(
                        pcd[:, c * NG, 0:4],
                        pcd[:, c * NG, 0:4],
                        s_t[:, c : c + 1],
                        None,
                        OP.mult,
                    )

                # --- Q[b, cg, i, j] = pab x pcd (one op, 512 cols) ---
                q = wpool.tile([TILE_B, CG, 4, 4], F32, tag="q")
                pab_b = pab[:].unsqueeze(3).broadcast_to([TILE_B, CG, 4, 4])
                pcd_b = pcd[:].unsqueeze(2).broadcast_to([TILE_B, CG, 4, 4])
                nc.vector.tensor_tensor(q[:], pab_b, pcd_b, OP.mult)

                # --- transpose Q (one [128,128] per chunk) -> wide PSUM ---
                qf = q[:].rearrange("p cg i j -> p (cg i j)")  # [128, 2048]
                ps_a = tps.tile([128, MACRO_B], F32, tag="ps_a")
                for c in range(CHUNK):
                    cw = slice(c * TILE_B, (c + 1) * TILE_B)
                    nc.tensor.transpose(
                        ps_a[:, cw], qf[:, c * 128 : (c + 1) * 128], id_sb[:]
                    )

                # --- copy QT halves to SBUF (2 wide ScalarE copies) ---
                # qts[t] rows: groups 4t..4t+3, 16 monomial rows each
                qts = [
                    qpool.tile([64, MACRO_B], F32, tag=f"qt{h}", name=f"qt{h}")
                    for h in range(2)
                ]
                nc.scalar.copy(qts[0][:], ps_a[0:64, :])
                nc.scalar.copy(qts[1][:], ps_a[64:128, :])

                # --- 8 group matmuls (K=32) + product chain ---
                # even groups: PSUM->SBUF copy on ScalarE; odd groups:
                # DVE multiplies PSUM x SBUF; GPSIMD folds the SBUF tree.
                g_ps = [
                    gps.tile([RU, MACRO_B], F32, tag=f"g{i}", name=f"g{i}")
                    for i in range(2)
                ]
                a_sb = [
                    qpool.tile([RU, MACRO_B], F32, tag=f"a{i}", name=f"a{i}")
                    for i in range(4)
                ]
                t_sb = [
                    qpool.tile([RU, MACRO_B], F32, tag=f"t{i}", name=f"t{i}")
                    for i in range(4)
                ]
                u_sb = [
                    qpool.tile([RU, MACRO_B], F32, tag=f"u{i}", name=f"u{i}")
                    for i in range(2)
                ]
                prod = qpool.tile([RU, MACRO_B], F32, tag="prod")
                for g in range(NG):
                    h, k = g // 2, g % 2
                    qt = qts[g // 4]
                    go = 32 * ((g % 4) // 2)  # == 32*(h%2)
                    csb = c_sb[h // 2]
                    dst = g_ps[g % 2]
                    nc.tensor.matmul(
                        dst[:],
                        csb[go : go + 32, RU * k : RU * (k + 1)],
                        qt[go : go + 32, :],
                        start=True,
                        stop=True,
                    )
                    # even groups: evacuate PSUM on ScalarE; odd: DVE mult
                    if g % 2 == 0:
                        nc.scalar.copy(a_sb[g // 2][:], dst[:])
                    else:
                        nc.vector.tensor_mul(
                            t_sb[g // 2][:], a_sb[g // 2][:], dst[:]
                        )
                nc.vector.tensor_mul(u_sb[0][:], t_sb[0][:], t_sb[1][:])
                nc.gpsimd.tensor_mul(u_sb[1][:], t_sb[2][:], t_sb[3][:])
                nc.vector.tensor_mul(prod[:], u_sb[0][:], u_sb[1][:])

                # --- sum over rank: out[u, b] = rind.T @ prod ---
                o_ps = ops_.tile([U, MACRO_B], F32, tag="o_ps")
                nc.tensor.matmul(o_ps[:], ri_sb[:], prod[:], start=True, stop=True)
                o_sb = qpool.tile([U, MACRO_B], F32, tag="o_sb")
                nc.scalar.copy(o_sb[:], o_ps[:])
                nc.sync.dma_start(out=out[mi], in_=o_sb[:])
    nc.finalize()
    return nc


def _pack_weights(kernel: np.ndarray):
    K = kernel.astype(np.float32)  # [2, R, F, U]
    C = np.zeros((128, 2 * RU), np.float32)
    bits = [(0, 0), (1, 0), (0, 1), (1, 1)]
    for g in range(NG):
        h, k = g // 2, g % 2
        r0 = 64 * (h // 2) + 32 * (h % 2) + 16 * k
        c0 = RU * k
        fs = [4 * g, 4 * g + 1, 4 * g + 2, 4 * g + 3]
        for i, (ba, bb) in enumerate(bits):
            for j, (bc, bd) in enumerate(bits):
                coef = (
                    K[ba, :, fs[0], :]
                    * K[bb, :, fs[1], :]
                    * K[bc, :, fs[2], :]
                    * K[bd, :, fs[3], :]
                )  # [R, U]
                C[r0 + i * 4 + j, c0 : c0 + RU] = coef.reshape(RU)
    ident = np.eye(128, dtype=np.float32)
    rind = np.zeros((RU, U), np.float32)
    for r in range(R):
        for u in range(U):
            rind[r * U + u, u] = 1.0
    return C, ident, rind


_NC_CACHE = {}


def kernel(X: np.ndarray, kernel: np.ndarray) -> np.ndarray:
    if "nc" not in _NC_CACHE:
        _NC_CACHE["nc"] = build_nc()
    nc = _NC_CACHE["nc"]
    C, ident, rind = _pack_weights(kernel)
    X = np.ascontiguousarray(X, dtype=np.float32)
    # [core, macro, chunk, partition, F] -> [core, macro, partition, chunk, F]
    Xd = (
        X.reshape(N_CORES, N_MACRO, CHUNK, TILE_B, F)
        .transpose(0, 1, 3, 2, 4)
        .copy()
    )
    in_maps = []
    for c in range(N_CORES):
        in_maps.append(
            {
                "X": Xd[c],
                "C": C,
                "ident": ident,
                "rind": rind,
            }
        )
    res = run_bass_kernel_spmd(nc, in_maps, core_ids=list(range(N_CORES)))
    outs = []
    for c in range(N_CORES):
        o = res.results[c]["out"]  # [N_MACRO, U, MACRO_B]
        outs.append(o.transpose(0, 2, 1).reshape(B_CORE, U))
    return np.concatenate(outs, axis=0).astype(np.float32)


if __name__ == "__main__":
    rng = np.random.default_rng(0)
    X = rng.standard_normal((B_FULL, F), dtype=np.float32)
    K = (rng.standard_normal((2, R, F, U)) * 0.24).astype(np.float32)
    y = kernel(X, K)
    print(y.shape, y.dtype, np.abs(y).max())



# revision 2
# speedup vs baseline: 1.4187x; 1.4187x over previous
"""Trainium2 Bass kernel for nn_CP_Based (CP-decomposition feature-product layer).

Math: out[b,u] = sum_r prod_f ( x0[b,f]*K[0,r,f,u] + x1[b,f]*K[1,r,f,u] )
  with x0 = 1/sqrt(1+X^2), x1 = X/sqrt(1+X^2).
Factor the normalization out of the f-product:
  out[b,u] = S[b] * sum_r prod_f ( K0[f,ru] + X[b,f]*K1[f,ru] ),
  S[b] = 1/sqrt(prod_f (1+X[b,f]^2)).
The 32-feature product is decomposed into 8 groups of 4 features. Each group's
product is a linear map from the 16 multilinear monomials of its 4 features:
  G[b, g, ru] = sum_m Q[b, g, m] * C[g, m, ru]
Layout: batch rows sit on the PARTITION axis of the matmul OUTPUT, so each
matmul is (stationary QT[m, b-chunk]) x (moving C-block[m, (g,ru)]):
  - Q [128b, (c,g,i,j)] built on DVE/GPSIMD from monomial halves, stored bf16
  - QT via one DMA-transpose instruction (no PE transpose, no PSUM evacuation)
  - 2 matmuls per 128-row chunk: groups 0-3 (K=64) and groups 4-7 (K=64),
    each out [128, 320] into its own PSUM bank, bf16 moving = 1 cycle/row
  - product over 8 groups = 3-level elementwise chain (bf16, DVE 2x mode),
    level 1 reads the two PSUM banks directly
  - sum over rank r: strided tensor_reduce (ru packed u-major: ru = u*10+r)
  - S computed on the Activation engine (Square, Ln(1+.), Exp(-0.5 .)) and
    applied to the final fp32 [128, 4, 8] tile.

Sharding: pure data-parallel over batch: 131072 rows -> 8 cores x 16384.
"""

import sys

import numpy as np

sys.path.insert(0, "/opt/trn_rl_repo")

import concourse.bacc as bacc  # noqa: E402
import concourse.mybir as mybir  # noqa: E402
from concourse.bass_utils import run_bass_kernel_spmd  # noqa: E402
from concourse.tile import TileContext  # noqa: E402

F32 = mybir.dt.float32
BF16 = mybir.dt.bfloat16
AF = mybir.ActivationFunctionType
OP = mybir.AluOpType
AX = mybir.AxisListType

B_FULL = 131072
N_CORES = 8
B_CORE = B_FULL // N_CORES  # 16384
F = 32
R, U = 10, 8
RU = R * U  # 80
NG = 8  # feature groups of 4
TILE_B = 128
CHUNK = 4  # 128-row chunks per macro tile
MACRO_B = TILE_B * CHUNK  # 512
N_MACRO = B_CORE // MACRO_B  # 32
CG = CHUNK * NG  # 32 (chunk, group) pairs


def build_nc():
    nc = bacc.Bacc()
    X = nc.dram_tensor(
        "X", [N_MACRO, TILE_B, CHUNK, F], F32, kind="ExternalInput"
    )
    # C rows: m = g*16 + i*4 + j; cols: g*80 + u*10 + r (within-half blocks)
    C = nc.dram_tensor("C", [128, 2 * 4 * RU], BF16, kind="ExternalInput")
    out = nc.dram_tensor(
        "out", [N_MACRO, TILE_B, CHUNK, U], F32, kind="ExternalOutput"
    )

    with TileContext(nc) as tc:
        with (
            tc.tile_pool(name="const", bufs=1) as cpool,
            tc.tile_pool(name="xin", bufs=3) as xpool,
            tc.tile_pool(name="sno", bufs=2) as spool,
            tc.tile_pool(name="mono", bufs=2) as mpool,
            tc.tile_pool(name="qq", bufs=2) as qpool,
            tc.tile_pool(name="qt", bufs=2) as tpool,
            tc.tile_pool(name="chain", bufs=2) as lpool,
            tc.tile_pool(name="outp", bufs=2) as opool,
            tc.tile_pool(name="psum", bufs=1, space="PSUM") as pspool,
        ):
            c_sb = cpool.tile([128, 2 * 4 * RU], BF16, tag="c_sb")
            nc.sync.dma_start(out=c_sb[:], in_=C[:, :])

            for mi in range(N_MACRO):
                # ---- load X for 512 rows: [128 part, 4 chunk, 32 f] ----
                xm = xpool.tile([TILE_B, CHUNK, F], F32, tag="x")
                nc.sync.dma_start(out=xm[:], in_=X[mi])

                # ---- S = exp(-0.5 * sum_f ln(1 + x^2)) on Act engine ----
                sq = spool.tile([TILE_B, CHUNK, F], F32, tag="sq")
                nc.scalar.activation(sq[:], xm[:], AF.Square)
                lnv = spool.tile([TILE_B, CHUNK, F], F32, tag="lnv")
                nc.scalar.activation(lnv[:], sq[:], AF.Ln, bias=1.0)
                lns = spool.tile([TILE_B, CHUNK], F32, tag="lns")
                nc.vector.tensor_reduce(lns[:], lnv[:], AX.X, OP.add)
                s_t = spool.tile([TILE_B, CHUNK], F32, tag="s_t")
                nc.scalar.activation(s_t[:], lns[:], AF.Exp, scale=-0.5)

                # ---- monomial halves pq[128, cg, 8]:
                #      [0:4]=(1,Xa,Xb,XaXb)  [4:8]=(1,Xc,Xd,XcXd) ----
                pq = mpool.tile([TILE_B, CG, 8], F32, tag="pq")
                if mi < 2:
                    # ones slots are never overwritten; write each of the 2
                    # rotating buffers once
                    ones = pq[:, :, 0:8:4]
                    nc.vector.memset(ones, 1.0)
                xg = xm[:].rearrange("p c (g j) -> p (c g) j", j=4)
                # slots (1,2) <- (Xa,Xb); (5,6) <- (Xc,Xd): one strided copy
                dst = pq[:].rearrange("p cg (h s) -> p cg h s", h=2)[
                    :, :, :, 1:3
                ]
                src = xg[:].rearrange("p cg (h s) -> p cg h s", h=2)
                nc.gpsimd.tensor_copy(dst, src)
                # slots (3,7) <- (Xa*Xb, Xc*Xd)
                dstm = pq[:].rearrange("p cg (h s) -> p cg h s", h=2)[
                    :, :, :, 3:4
                ]
                nc.gpsimd.tensor_tensor(
                    dstm,
                    src[:, :, :, 0:1],
                    src[:, :, :, 1:2],
                    OP.mult,
                )

                # ---- outer product Q[b, cg, i, j] (bf16 out) ----
                q = qpool.tile([TILE_B, CG, 4, 4], BF16, tag="q")
                pab_b = (
                    pq[:, :, 0:4]
                    .unsqueeze(3)
                    .broadcast_to([TILE_B, CG, 4, 4])
                )
                pcd_b = (
                    pq[:, :, 4:8]
                    .unsqueeze(2)
                    .broadcast_to([TILE_B, CG, 4, 4])
                )
                nc.vector.tensor_tensor(
                    q[:, 0:16], pab_b[:, 0:16], pcd_b[:, 0:16], OP.mult
                )
                nc.gpsimd.tensor_tensor(
                    q[:, 16:32], pab_b[:, 16:32], pcd_b[:, 16:32], OP.mult
                )

                # ---- transpose: qt[m, c, b] = q[b, (c, m)] ----
                qt = tpool.tile([128, CHUNK, TILE_B], BF16, tag="qt")
                nc.sync.dma_start_transpose(
                    qt[:], q[:].rearrange("p cg i j -> p (cg i j)")
                )

                # ---- per chunk: 2 matmuls -> PSUM banks A (g0-3), B (g4-7) --
                psA = []
                psB = []
                for c in range(CHUNK):
                    pa = pspool.tile([128, 512], F32, tag=f"pa{c}", name=f"pa{c}")
                    pb = pspool.tile([128, 512], F32, tag=f"pb{c}", name=f"pb{c}")
                    nc.tensor.matmul(
                        pa[:, 0:320],
                        qt[0:64, c, :],
                        c_sb[0:64, 0:320],
                        start=True,
                        stop=True,
                    )
                    nc.tensor.matmul(
                        pb[:, 0:320],
                        qt[64:128, c, :],
                        c_sb[64:128, 320:640],
                        start=True,
                        stop=True,
                    )
                    psA.append(pa)
                    psB.append(pb)

                # ---- product chain over groups (bf16) ----
                l1 = lpool.tile([TILE_B, CHUNK, 4, RU], BF16, tag="l1")
                for c in range(CHUNK):
                    eng = nc.vector if c < 2 else nc.gpsimd
                    eng.tensor_tensor(
                        l1[:, c],
                        psA[c][:, 0:320].rearrange("p (g k) -> p g k", g=4),
                        psB[c][:, 0:320].rearrange("p (g k) -> p g k", g=4),
                        OP.mult,
                    )
                l2 = lpool.tile([TILE_B, CHUNK, 2, RU], BF16, tag="l2")
                nc.vector.tensor_tensor(
                    l2[:], l1[:, :, 0:2], l1[:, :, 2:4], OP.mult
                )
                l3 = lpool.tile([TILE_B, CHUNK, RU], BF16, tag="l3")
                nc.vector.tensor_tensor(
                    l3[:], l2[:, :, 0], l2[:, :, 1], OP.mult
                )

                # ---- sum over rank (ru packed u-major: ru = u*10 + r) ----
                of = opool.tile([TILE_B, CHUNK, U], F32, tag="of")
                nc.vector.tensor_reduce(
                    of[:],
                    l3[:].rearrange("p c (u r) -> p c u r", r=R),
                    AX.X,
                    OP.add,
                )
                # ---- apply S ----
                os_ = opool.tile([TILE_B, CHUNK, U], F32, tag="os")
                nc.vector.tensor_tensor(
                    os_[:],
                    of[:],
                    s_t[:].unsqueeze(2).broadcast_to([TILE_B, CHUNK, U]),
                    OP.mult,
                )
                nc.sync.dma_start(out=out[mi], in_=os_[:])
    nc.finalize()
    return nc


def _pack_weights(kernel: np.ndarray):
    import ml_dtypes

    K = kernel.astype(np.float64)  # [2, R, F, U]
    C = np.zeros((128, 2 * 4 * RU), np.float64)
    bits = [(0, 0), (1, 0), (0, 1), (1, 1)]
    for g in range(NG):
        half = g // 4
        for i, (ba, bb) in enumerate(bits):
            for j, (bc, bd) in enumerate(bits):
                m = g * 16 + i * 4 + j
                coef = (
                    K[ba, :, 4 * g, :]
                    * K[bb, :, 4 * g + 1, :]
                    * K[bc, :, 4 * g + 2, :]
                    * K[bd, :, 4 * g + 3, :]
                )  # [R, U]
                col0 = half * 320 + (g % 4) * RU
                # ru = u*10 + r
                C[m, col0 : col0 + RU] = coef.T.reshape(RU)
    return C.astype(ml_dtypes.bfloat16)


_NC_CACHE = {}


def kernel(X: np.ndarray, kernel: np.ndarray) -> np.ndarray:
    if "nc" not in _NC_CACHE:
        _NC_CACHE["nc"] = build_nc()
    nc = _NC_CACHE["nc"]
    C = _pack_weights(kernel)
    X = np.ascontiguousarray(X, dtype=np.float32)
    # [core, macro, chunk, partition, F] -> [core, macro, partition, chunk, F]
    Xd = (
        X.reshape(N_CORES, N_MACRO, CHUNK, TILE_B, F)
        .transpose(0, 1, 3, 2, 4)
        .copy()
    )
    in_maps = []
    for c in range(N_CORES):
        in_maps.append({"X": Xd[c], "C": C})
    res = run_bass_kernel_spmd(nc, in_maps, core_ids=list(range(N_CORES)))
    outs = []
    for c in range(N_CORES):
        o = res.results[c]["out"]  # [N_MACRO, TILE_B, CHUNK, U]
        outs.append(o.transpose(0, 2, 1, 3).reshape(B_CORE, U))
    return np.concatenate(outs, axis=0).astype(np.float32)


if __name__ == "__main__":
    rng = np.random.default_rng(0)
    X = rng.standard_normal((B_FULL, F), dtype=np.float32)
    K = (rng.standard_normal((2, R, F, U)) * 0.24).astype(np.float32)
    y = kernel(X, K)
    print(y.shape, y.dtype, np.abs(y).max())


# revision 6
# speedup vs baseline: 1.6725x; 1.1789x over previous
"""Trainium2 Bass kernel for nn_CP_Based (CP-decomposition feature-product layer).

Math: out[b,u] = sum_r prod_f ( x0[b,f]*K[0,r,f,u] + x1[b,f]*K[1,r,f,u] )
  with x0 = 1/sqrt(1+X^2), x1 = X/sqrt(1+X^2).
Factor the normalization out of the f-product:
  out[b,u] = S[b] * sum_r prod_f ( K0[f,ru] + X[b,f]*K1[f,ru] ),
  S[b] = 1/sqrt(prod_f (1+X[b,f]^2)).
The 32-feature product is decomposed into 8 groups of 4 features. Each group's
product is a linear map from the 16 multilinear monomials of its 4 features:
  G[b, g, ru] = sum_m Q[b, g, m] * C[g, m, ru]
Layout: batch rows sit on the PARTITION axis of the matmul OUTPUT, so each
matmul is (stationary QT[m, b-chunk]) x (moving C-block[m, (g,ru)]):
  - Q [128b, (c,g,i,j)] built on DVE/GPSIMD from monomial halves, stored bf16
  - QT via one DMA-transpose instruction (no PE transpose, no PSUM evacuation)
  - 2 matmuls per 128-row chunk: groups 0-3 (K=64) and groups 4-7 (K=64),
    each out [128, 320] into its own PSUM bank, bf16 moving = 1 cycle/row
  - product over 8 groups = 3-level elementwise chain (bf16, DVE 2x mode),
    level 1 reads the two PSUM banks directly
  - sum over rank r: strided tensor_reduce (ru packed u-major: ru = u*10+r)
  - S computed on the Activation engine (Square, Ln(1+.), Exp(-0.5 .)) and
    applied to the final fp32 [128, 4, 8] tile.

Sharding: pure data-parallel over batch: 131072 rows -> 8 cores x 16384.
"""

import sys

import numpy as np

sys.path.insert(0, "/opt/trn_rl_repo")

import concourse.bacc as bacc  # noqa: E402
import concourse.mybir as mybir  # noqa: E402
from concourse.bass_utils import run_bass_kernel_spmd  # noqa: E402
from concourse.tile import TileContext  # noqa: E402

F32 = mybir.dt.float32
BF16 = mybir.dt.bfloat16
AF = mybir.ActivationFunctionType
OP = mybir.AluOpType
AX = mybir.AxisListType

B_FULL = 131072
N_CORES = 8
B_CORE = B_FULL // N_CORES  # 16384
F = 32
R, U = 10, 8
RU = R * U  # 80
NG = 8  # feature groups of 4
TILE_B = 128
CHUNK = 4  # 128-row chunks per macro tile
MACRO_B = TILE_B * CHUNK  # 512
N_MACRO = B_CORE // MACRO_B  # 32
CG = CHUNK * NG  # 32 (chunk, group) pairs


def build_nc():
    nc = bacc.Bacc()
    X = nc.dram_tensor(
        "X", [N_MACRO, TILE_B, CHUNK, F], F32, kind="ExternalInput"
    )
    # C rows: m = g*16 + i*4 + j; cols: g*80 + u*10 + r (within-half blocks)
    C = nc.dram_tensor("C", [128, 2 * 4 * RU], BF16, kind="ExternalInput")
    out = nc.dram_tensor(
        "out", [N_MACRO, TILE_B, CHUNK, U], F32, kind="ExternalOutput"
    )

    with TileContext(nc) as tc:
        with (
            tc.tile_pool(name="const", bufs=1) as cpool,
            tc.tile_pool(name="xin", bufs=4) as xpool,
            tc.tile_pool(name="sno", bufs=4) as spool,
            tc.tile_pool(name="mono", bufs=3) as mpool,
            tc.tile_pool(name="qq", bufs=3) as qpool,
            tc.tile_pool(name="qt", bufs=3) as tpool,
            tc.tile_pool(name="chain", bufs=3) as lpool,
            tc.tile_pool(name="outp", bufs=3) as opool,
            tc.tile_pool(name="psum", bufs=1, space="PSUM") as pspool,
        ):
            c_sb = cpool.tile([128, 2 * 4 * RU], BF16, tag="c_sb")
            nc.sync.dma_start(out=c_sb[:], in_=C[:, :])

            for mi in range(N_MACRO):
                # ---- load X for 512 rows: [128 part, 4 chunk, 32 f] ----
                xm = xpool.tile([TILE_B, CHUNK, F], F32, tag="x")
                nc.sync.dma_start(out=xm[:], in_=X[mi])

                # ---- S = rsqrt(prod_f (1 + x^2)); Square and Rsqrt share
                #      one act table set, so no per-macro table swaps ----
                sq = spool.tile([TILE_B, CHUNK, F], F32, tag="sq")
                nc.scalar.activation(sq[:], xm[:], AF.Square)
                sp1 = spool.tile([TILE_B, CHUNK, F], F32, tag="sp1")
                nc.vector.tensor_scalar_add(sp1[:], sq[:], 1.0)
                pr = spool.tile([TILE_B, CHUNK], F32, tag="pr")
                nc.vector.tensor_reduce(pr[:], sp1[:], AX.X, OP.mult)
                rp = spool.tile([TILE_B, CHUNK], F32, tag="rp")
                nc.vector.reciprocal(rp[:], pr[:])
                s_t = spool.tile([TILE_B, CHUNK], F32, tag="s_t")
                nc.scalar.activation(s_t[:], rp[:], AF.Sqrt)

                # ---- monomial halves pq[128, cg, 8]:
                #      [0:4]=(1,Xa,Xb,XaXb)  [4:8]=(1,Xc,Xd,XcXd) ----
                pq = mpool.tile([TILE_B, CG, 8], F32, tag="pq")
                if mi < 3:
                    # ones slots are never overwritten; write each of the 2
                    # rotating buffers once
                    ones = pq[:, :, 0:8:4]
                    nc.vector.memset(ones, 1.0)
                xg = xm[:].rearrange("p c (g j) -> p (c g) j", j=4)
                # slots (1,2) <- (Xa,Xb); (5,6) <- (Xc,Xd): one strided copy
                dst = pq[:].rearrange("p cg (h s) -> p cg h s", h=2)[
                    :, :, :, 1:3
                ]
                src = xg[:].rearrange("p cg (h s) -> p cg h s", h=2)
                nc.gpsimd.tensor_copy(dst, src)
                # slots (3,7) <- (Xa*Xb, Xc*Xd)
                dstm = pq[:].rearrange("p cg (h s) -> p cg h s", h=2)[
                    :, :, :, 3:4
                ]
                nc.gpsimd.tensor_tensor(
                    dstm,
                    src[:, :, :, 0:1],
                    src[:, :, :, 1:2],
                    OP.mult,
                )

                # ---- outer product Q[b, cg, i, j] (bf16 out) ----
                q = qpool.tile([TILE_B, CG, 4, 4], BF16, tag="q")
                pab_b = (
                    pq[:, :, 0:4]
                    .unsqueeze(3)
                    .broadcast_to([TILE_B, CG, 4, 4])
                )
                pcd_b = (
                    pq[:, :, 4:8]
                    .unsqueeze(2)
                    .broadcast_to([TILE_B, CG, 4, 4])
                )
                nc.vector.tensor_tensor(
                    q[:, 0:16], pab_b[:, 0:16], pcd_b[:, 0:16], OP.mult
                )
                nc.gpsimd.tensor_tensor(
                    q[:, 16:32], pab_b[:, 16:32], pcd_b[:, 16:32], OP.mult
                )

                # ---- transpose: qt[m, c, b] = q[b, (c, m)] ----
                qt = tpool.tile([128, CHUNK, TILE_B], BF16, tag="qt")
                nc.sync.dma_start_transpose(
                    qt[:], q[:].rearrange("p cg i j -> p (cg i j)")
                )

                # ---- per chunk: 2 matmuls -> PSUM banks A (g0-3), B (g4-7) --
                psA = []
                psB = []
                for c in range(CHUNK):
                    pa = pspool.tile([128, 512], F32, tag=f"pa{c}", name=f"pa{c}")
                    pb = pspool.tile([128, 512], F32, tag=f"pb{c}", name=f"pb{c}")
                    nc.tensor.matmul(
                        pa[:, 0:320],
                        qt[0:64, c, :],
                        c_sb[0:64, 0:320],
                        start=True,
                        stop=True,
                    )
                    nc.tensor.matmul(
                        pb[:, 0:320],
                        qt[64:128, c, :],
                        c_sb[64:128, 320:640],
                        start=True,
                        stop=True,
                    )
                    psA.append(pa)
                    psB.append(pb)

                # ---- product chain over groups (bf16) ----
                l1 = lpool.tile([TILE_B, CHUNK, 4, RU], BF16, tag="l1")
                for c in range(CHUNK):
                    eng = nc.vector if c < 2 else nc.gpsimd
                    eng.tensor_tensor(
                        l1[:, c],
                        psA[c][:, 0:320].rearrange("p (g k) -> p g k", g=4),
                        psB[c][:, 0:320].rearrange("p (g k) -> p g k", g=4),
                        OP.mult,
                    )
                l2 = lpool.tile([TILE_B, CHUNK, 2, RU], BF16, tag="l2")
                nc.vector.tensor_tensor(
                    l2[:], l1[:, :, 0:2], l1[:, :, 2:4], OP.mult
                )
                l3 = lpool.tile([TILE_B, CHUNK, RU], BF16, tag="l3")
                nc.vector.tensor_tensor(
                    l3[:], l2[:, :, 0], l2[:, :, 1], OP.mult
                )

                # ---- sum over rank (ru packed u-major: ru = u*10 + r) ----
                of = opool.tile([TILE_B, CHUNK, U], F32, tag="of")
                nc.vector.tensor_reduce(
                    of[:],
                    l3[:].rearrange("p c (u r) -> p c u r", r=R),
                    AX.X,
                    OP.add,
                )
                # ---- apply S ----
                os_ = opool.tile([TILE_B, CHUNK, U], F32, tag="os")
                nc.vector.tensor_tensor(
                    os_[:],
                    of[:],
                    s_t[:].unsqueeze(2).broadcast_to([TILE_B, CHUNK, U]),
                    OP.mult,
                )
                nc.sync.dma_start(out=out[mi], in_=os_[:])
    nc.finalize()
    return nc


def _pack_weights(kernel: np.ndarray):
    import ml_dtypes

    K = kernel.astype(np.float64)  # [2, R, F, U]
    C = np.zeros((128, 2 * 4 * RU), np.float64)
    bits = [(0, 0), (1, 0), (0, 1), (1, 1)]
    for g in range(NG):
        half = g // 4
        for i, (ba, bb) in enumerate(bits):
            for j, (bc, bd) in enumerate(bits):
                m = g * 16 + i * 4 + j
                coef = (
                    K[ba, :, 4 * g, :]
                    * K[bb, :, 4 * g + 1, :]
                    * K[bc, :, 4 * g + 2, :]
                    * K[bd, :, 4 * g + 3, :]
                )  # [R, U]
                col0 = half * 320 + (g % 4) * RU
                # ru = u*10 + r
                C[m, col0 : col0 + RU] = coef.T.reshape(RU)
    return C.astype(ml_dtypes.bfloat16)


_NC_CACHE = {}


def kernel(X: np.ndarray, kernel: np.ndarray) -> np.ndarray:
    if "nc" not in _NC_CACHE:
        _NC_CACHE["nc"] = build_nc()
    nc = _NC_CACHE["nc"]
    C = _pack_weights(kernel)
    X = np.ascontiguousarray(X, dtype=np.float32)
    # [core, macro, chunk, partition, F] -> [core, macro, partition, chunk, F]
    Xd = (
        X.reshape(N_CORES, N_MACRO, CHUNK, TILE_B, F)
        .transpose(0, 1, 3, 2, 4)
        .copy()
    )
    in_maps = []
    for c in range(N_CORES):
        in_maps.append({"X": Xd[c], "C": C})
    res = run_bass_kernel_spmd(nc, in_maps, core_ids=list(range(N_CORES)))
    outs = []
    for c in range(N_CORES):
        o = res.results[c]["out"]  # [N_MACRO, TILE_B, CHUNK, U]
        outs.append(o.transpose(0, 2, 1, 3).reshape(B_CORE, U))
    return np.concatenate(outs, axis=0).astype(np.float32)


if __name__ == "__main__":
    rng = np.random.default_rng(0)
    X = rng.standard_normal((B_FULL, F), dtype=np.float32)
    K = (rng.standard_normal((2, R, F, U)) * 0.24).astype(np.float32)
    y = kernel(X, K)
    print(y.shape, y.dtype, np.abs(y).max())


# revision 7
# speedup vs baseline: 1.9661x; 1.1755x over previous
"""Trainium2 Bass kernel for nn_CP_Based (CP-decomposition feature-product layer).

Math: out[b,u] = sum_r prod_f ( x0[b,f]*K[0,r,f,u] + x1[b,f]*K[1,r,f,u] )
  with x0 = 1/sqrt(1+X^2), x1 = X/sqrt(1+X^2).
Factor the normalization out of the f-product:
  out[b,u] = S[b] * sum_r prod_f ( K0[f,ru] + X[b,f]*K1[f,ru] ),
  S[b] = 1/sqrt(prod_f (1+X[b,f]^2)).
The 32-feature product is decomposed into 8 groups of 4 features. Each group's
product is a linear map from the 16 multilinear monomials of its 4 features:
  G[b, g, ru] = sum_m Q[b, g, m] * C[g, m, ru]
Layout: batch rows sit on the PARTITION axis of the matmul OUTPUT, so each
matmul is (stationary QT[m, b-chunk]) x (moving C-block[m, (g,ru)]):
  - Q [128b, (c,g,i,j)] built on DVE/GPSIMD from monomial halves, stored bf16
  - QT via one DMA-transpose instruction (no PE transpose, no PSUM evacuation)
  - 2 matmuls per 128-row chunk: groups 0-3 (K=64) and groups 4-7 (K=64),
    each out [128, 320] into its own PSUM bank, bf16 moving = 1 cycle/row
  - product over 8 groups = 3-level elementwise chain (bf16, DVE 2x mode),
    level 1 reads the two PSUM banks directly
  - sum over rank r: strided tensor_reduce (ru packed u-major: ru = u*10+r)
  - S computed via Act Square + DVE (+1, prod-reduce, reciprocal) + Act Sqrt
    (Square and Sqrt share one act table set -> no per-macro table reloads)
The emission is software-pipelined one stage deep: each iteration emits the
pre-matmul front-end of macro m, then the post-matmul chain of macro m-1,
then the matmuls of macro m, so no engine queue head-of-line blocks on the
PSUM->chain dependency. X loads and output stores are batched 4 macros per
DMA to keep HWDGE occupancy low.

Sharding: pure data-parallel over batch: 131072 rows -> 8 cores x 16384.
"""

import sys

import numpy as np

sys.path.insert(0, "/opt/trn_rl_repo")

import concourse.bacc as bacc  # noqa: E402
import concourse.mybir as mybir  # noqa: E402
from concourse.bass_utils import run_bass_kernel_spmd  # noqa: E402
from concourse.tile import TileContext  # noqa: E402

F32 = mybir.dt.float32
BF16 = mybir.dt.bfloat16
AF = mybir.ActivationFunctionType
OP = mybir.AluOpType
AX = mybir.AxisListType

B_FULL = 131072
N_CORES = 8
B_CORE = B_FULL // N_CORES  # 16384
F = 32
R, U = 10, 8
RU = R * U  # 80
NG = 8  # feature groups of 4
TILE_B = 128
CHUNK = 4  # 128-row chunks per macro tile
MACRO_B = TILE_B * CHUNK  # 512
N_MACRO = B_CORE // MACRO_B  # 32
CG = CHUNK * NG  # 32 (chunk, group) pairs
GRP = 4  # macros per X-load / out-store DMA
N_GRP = N_MACRO // GRP  # 8


def build_nc():
    nc = bacc.Bacc()
    X = nc.dram_tensor(
        "X", [N_GRP, TILE_B, GRP, CHUNK, F], F32, kind="ExternalInput"
    )
    # C rows: m = g*16 + i*4 + j; cols: g*80 + u*10 + r (within-half blocks)
    C = nc.dram_tensor("C", [128, 2 * 4 * RU], BF16, kind="ExternalInput")
    out = nc.dram_tensor(
        "out", [N_GRP, TILE_B, GRP, CHUNK, U], F32, kind="ExternalOutput"
    )

    with TileContext(nc) as tc:
        with (
            tc.tile_pool(name="const", bufs=1) as cpool,
            tc.tile_pool(name="xin", bufs=2) as xpool,
            tc.tile_pool(name="sno", bufs=3) as spool,
            tc.tile_pool(name="mono", bufs=3) as mpool,
            tc.tile_pool(name="qq", bufs=3) as qpool,
            tc.tile_pool(name="qt", bufs=3) as tpool,
            tc.tile_pool(name="chain", bufs=2) as lpool,
            tc.tile_pool(name="outp", bufs=2) as opool,
            tc.tile_pool(name="psum", bufs=1, space="PSUM") as pspool,
        ):
            c_sb = cpool.tile([128, 2 * 4 * RU], BF16, tag="c_sb")
            nc.sync.dma_start(out=c_sb[:], in_=C[:, :])

            state = {}  # macro index -> tiles needed by the back-end

            def front(mi):
                gi, k = divmod(mi, GRP)
                if k == 0:
                    xg_t = xpool.tile(
                        [TILE_B, GRP, CHUNK, F], F32, tag="x", name="xt"
                    )
                    nc.sync.dma_start(out=xg_t[:], in_=X[gi])
                    state["xg"] = xg_t
                xm = state["xg"][:, k]  # [128, CHUNK, F]

                # ---- S = 1/sqrt(prod_f (1 + x^2)) ----
                sq = spool.tile([TILE_B, CHUNK, F], F32, tag="sq")
                nc.scalar.activation(sq[:], xm, AF.Square)
                sp1 = spool.tile([TILE_B, CHUNK, F], F32, tag="sp1")
                nc.vector.tensor_scalar_add(sp1[:], sq[:], 1.0)
                pr = spool.tile([TILE_B, CHUNK], F32, tag="pr")
                nc.vector.tensor_reduce(pr[:], sp1[:], AX.X, OP.mult)
                rp = spool.tile([TILE_B, CHUNK], F32, tag="rp")
                nc.vector.reciprocal(rp[:], pr[:])
                s_t = spool.tile([TILE_B, CHUNK], F32, tag="s_t")
                nc.scalar.activation(s_t[:], rp[:], AF.Sqrt)

                # ---- monomial halves pq[128, cg, 8] ----
                pq = mpool.tile([TILE_B, CG, 8], F32, tag="pq")
                if mi < 3:
                    nc.vector.memset(pq[:, :, 0:8:4], 1.0)
                xg = xm.rearrange("p c (g j) -> p (c g) j", j=4)
                src = xg.rearrange("p cg (h s) -> p cg h s", h=2)
                dst = pq[:].rearrange("p cg (h s) -> p cg h s", h=2)[
                    :, :, :, 1:3
                ]
                nc.gpsimd.tensor_copy(dst, src)
                dstm = pq[:].rearrange("p cg (h s) -> p cg h s", h=2)[
                    :, :, :, 3:4
                ]
                nc.gpsimd.tensor_tensor(
                    dstm, src[:, :, :, 0:1], src[:, :, :, 1:2], OP.mult
                )

                # ---- outer product Q[b, cg, i, j] (bf16) ----
                q = qpool.tile([TILE_B, CG, 4, 4], BF16, tag="q")
                pab_b = (
                    pq[:, :, 0:4].unsqueeze(3).broadcast_to([TILE_B, CG, 4, 4])
                )
                pcd_b = (
                    pq[:, :, 4:8].unsqueeze(2).broadcast_to([TILE_B, CG, 4, 4])
                )
                nc.vector.tensor_tensor(
                    q[:, 0:16], pab_b[:, 0:16], pcd_b[:, 0:16], OP.mult
                )
                nc.gpsimd.tensor_tensor(
                    q[:, 16:32], pab_b[:, 16:32], pcd_b[:, 16:32], OP.mult
                )

                # ---- transpose: qt[m, c, b] = q[b, (c, m)] ----
                qt = tpool.tile([128, CHUNK, TILE_B], BF16, tag="qt")
                nc.sync.dma_start_transpose(
                    qt[:], q[:].rearrange("p cg i j -> p (cg i j)")
                )
                state[mi] = {"qt": qt, "s_t": s_t}

            def matmuls(mi):
                st = state[mi]
                qt = st["qt"]
                psA, psB = [], []
                for c in range(CHUNK):
                    pa = pspool.tile(
                        [128, 512], F32, tag=f"pa{c}", name=f"pa{c}"
                    )
                    pb = pspool.tile(
                        [128, 512], F32, tag=f"pb{c}", name=f"pb{c}"
                    )
                    nc.tensor.matmul(
                        pa[:, 0:320],
                        qt[0:64, c, :],
                        c_sb[0:64, 0:320],
                        start=True,
                        stop=True,
                    )
                    nc.tensor.matmul(
                        pb[:, 0:320],
                        qt[64:128, c, :],
                        c_sb[64:128, 320:640],
                        start=True,
                        stop=True,
                    )
                    psA.append(pa)
                    psB.append(pb)
                st["psA"], st["psB"] = psA, psB

            def back(mi):
                st = state.pop(mi)
                psA, psB, s_t = st["psA"], st["psB"], st["s_t"]
                gi, k = divmod(mi, GRP)

                l1 = lpool.tile([TILE_B, CHUNK, 4, RU], BF16, tag="l1")
                for c in range(CHUNK):
                    eng = nc.vector if c < 2 else nc.gpsimd
                    eng.tensor_tensor(
                        l1[:, c],
                        psA[c][:, 0:320].rearrange("p (g k) -> p g k", g=4),
                        psB[c][:, 0:320].rearrange("p (g k) -> p g k", g=4),
                        OP.mult,
                    )
                l2 = lpool.tile([TILE_B, CHUNK, 2, RU], BF16, tag="l2")
                nc.vector.tensor_tensor(
                    l2[:], l1[:, :, 0:2], l1[:, :, 2:4], OP.mult
                )
                l3 = lpool.tile([TILE_B, CHUNK, RU], BF16, tag="l3")
                nc.vector.tensor_tensor(
                    l3[:], l2[:, :, 0], l2[:, :, 1], OP.mult
                )

                # ---- sum over rank (ru = u*10 + r) ----
                of = opool.tile([TILE_B, CHUNK, U], F32, tag="of")
                nc.vector.tensor_reduce(
                    of[:],
                    l3[:].rearrange("p c (u r) -> p c u r", r=R),
                    AX.X,
                    OP.add,
                )
                # ---- apply S, into the grouped store tile ----
                if k == 0:
                    state["ost"] = opool.tile(
                        [TILE_B, GRP, CHUNK, U], F32, tag="os", name="ost"
                    )
                os_ = state["ost"]
                nc.vector.tensor_tensor(
                    os_[:, k],
                    of[:],
                    s_t[:].unsqueeze(2).broadcast_to([TILE_B, CHUNK, U]),
                    OP.mult,
                )
                if k == GRP - 1:
                    nc.sync.dma_start(out=out[gi], in_=os_[:])

            # software-pipelined emission: front(m), back(m-1), matmuls(m)
            front(0)
            matmuls(0)
            for mi in range(1, N_MACRO):
                front(mi)
                back(mi - 1)
                matmuls(mi)
            back(N_MACRO - 1)
    nc.finalize()
    return nc


def _pack_weights(kernel: np.ndarray):
    import ml_dtypes

    K = kernel.astype(np.float64)  # [2, R, F, U]
    C = np.zeros((128, 2 * 4 * RU), np.float64)
    bits = [(0, 0), (1, 0), (0, 1), (1, 1)]
    for g in range(NG):
        half = g // 4
        for i, (ba, bb) in enumerate(bits):
            for j, (bc, bd) in enumerate(bits):
                m = g * 16 + i * 4 + j
                coef = (
                    K[ba, :, 4 * g, :]
                    * K[bb, :, 4 * g + 1, :]
                    * K[bc, :, 4 * g + 2, :]
                    * K[bd, :, 4 * g + 3, :]
                )  # [R, U]
                col0 = half * 320 + (g % 4) * RU
                # ru = u*10 + r
                C[m, col0 : col0 + RU] = coef.T.reshape(RU)
    return C.astype(ml_dtypes.bfloat16)


_NC_CACHE = {}


def kernel(X: np.ndarray, kernel: np.ndarray) -> np.ndarray:
    if "nc" not in _NC_CACHE:
        _NC_CACHE["nc"] = build_nc()
    nc = _NC_CACHE["nc"]
    C = _pack_weights(kernel)
    X = np.ascontiguousarray(X, dtype=np.float32)
    # row b of core = gi*2048 + k*512 + c*128 + p  ->  [gi, p, k, c, f]
    Xd = (
        X.reshape(N_CORES, N_GRP, GRP, CHUNK, TILE_B, F)
        .transpose(0, 1, 4, 2, 3, 5)
        .copy()
    )
    in_maps = []
    for c in range(N_CORES):
        in_maps.append({"X": Xd[c], "C": C})
    res = run_bass_kernel_spmd(nc, in_maps, core_ids=list(range(N_CORES)))
    outs = []
    for c in range(N_CORES):
        o = res.results[c]["out"]  # [N_GRP, TILE_B, GRP, CHUNK, U]
        outs.append(o.transpose(0, 2, 3, 1, 4).reshape(B_CORE, U))
    return np.concatenate(outs, axis=0).astype(np.float32)


if __name__ == "__main__":
    rng = np.random.default_rng(0)
    X = rng.standard_normal((B_FULL, F), dtype=np.float32)
    K = (rng.standard_normal((2, R, F, U)) * 0.24).astype(np.float32)
    y = kernel(X, K)
    print(y.shape, y.dtype, np.abs(y).max())


# revision 8
# speedup vs baseline: 2.4319x; 1.2369x over previous
"""Trainium2 Bass kernel for nn_CP_Based (CP-decomposition feature-product layer).

Math: out[b,u] = sum_r prod_f ( x0[b,f]*K[0,r,f,u] + x1[b,f]*K[1,r,f,u] )
  with x0 = 1/sqrt(1+X^2), x1 = X/sqrt(1+X^2).
Factor the normalization out of the f-product:
  out[b,u] = S[b] * sum_r prod_f ( K0[f,ru] + X[b,f]*K1[f,ru] ),
  S[b] = 1/sqrt(prod_f (1+X[b,f]^2)).
The 32-feature product is decomposed into 8 groups of 4 features. Each group's
product is a linear map from the 16 multilinear monomials of its 4 features:
  G[b, g, ru] = sum_m Q[b, g, m] * C[g, m, ru]
Layout: batch rows sit on the PARTITION axis of the matmul OUTPUT, so each
matmul is (stationary QT[m, b-chunk]) x (moving C-block[m, (g,ru)]):
  - Q [128b, (c,g,i,j)] built on DVE/GPSIMD from monomial halves, stored bf16
  - QT via one DMA-transpose instruction (no PE transpose, no PSUM evacuation)
  - 2 matmuls per 128-row chunk: groups 0-3 (K=64) and groups 4-7 (K=64),
    each out [128, 320] into its own PSUM bank, bf16 moving = 1 cycle/row
  - product over 8 groups = 3-level elementwise chain (bf16, DVE 2x mode),
    level 1 reads the two PSUM banks directly
  - sum over rank r: strided tensor_reduce (ru packed u-major: ru = u*10+r)
  - S computed via Act Square + DVE (+1, prod-reduce, reciprocal) + Act Sqrt
    (Square and Sqrt share one act table set -> no per-macro table reloads)
The emission is software-pipelined one stage deep: each iteration emits the
pre-matmul front-end of macro m, then the post-matmul chain of macro m-1,
then the matmuls of macro m, so no engine queue head-of-line blocks on the
PSUM->chain dependency. X loads and output stores are batched 4 macros per
DMA to keep HWDGE occupancy low.

Sharding: pure data-parallel over batch: 131072 rows -> 8 cores x 16384.
"""

import sys

import numpy as np

sys.path.insert(0, "/opt/trn_rl_repo")

import concourse.bacc as bacc  # noqa: E402
import concourse.mybir as mybir  # noqa: E402
from concourse.bass_utils import run_bass_kernel_spmd  # noqa: E402
from concourse.tile import TileContext  # noqa: E402

F32 = mybir.dt.float32
BF16 = mybir.dt.bfloat16
AF = mybir.ActivationFunctionType
OP = mybir.AluOpType
AX = mybir.AxisListType

B_FULL = 131072
N_CORES = 8
B_CORE = B_FULL // N_CORES  # 16384
F = 32
R, U = 10, 8
RU = R * U  # 80
NG = 8  # feature groups of 4
TILE_B = 128
CHUNK = 4  # 128-row chunks per macro tile
MACRO_B = TILE_B * CHUNK  # 512
N_MACRO = B_CORE // MACRO_B  # 32
CG = CHUNK * NG  # 32 (chunk, group) pairs
GRP = 4  # macros per X-load / out-store DMA
N_GRP = N_MACRO // GRP  # 8


def build_nc():
    nc = bacc.Bacc()
    X = nc.dram_tensor(
        "X", [N_GRP, TILE_B, GRP, CHUNK, F], F32, kind="ExternalInput"
    )
    # C rows: m = g*16 + i*4 + j; cols: g*80 + u*10 + r (within-half blocks)
    C = nc.dram_tensor("C", [128, 2 * 4 * RU], BF16, kind="ExternalInput")
    out = nc.dram_tensor(
        "out", [N_GRP, TILE_B, GRP, CHUNK, U], F32, kind="ExternalOutput"
    )

    with TileContext(nc) as tc:
        with (
            tc.tile_pool(name="const", bufs=1) as cpool,
            tc.tile_pool(name="xin", bufs=2) as xpool,
            tc.tile_pool(name="sno", bufs=3) as spool,
            tc.tile_pool(name="mono", bufs=3) as mpool,
            tc.tile_pool(name="qq", bufs=3) as qpool,
            tc.tile_pool(name="qt", bufs=3) as tpool,
            tc.tile_pool(name="chain", bufs=2) as lpool,
            tc.tile_pool(name="outp", bufs=2) as opool,
            tc.tile_pool(name="psum", bufs=1, space="PSUM") as pspool,
        ):
            c_sb = cpool.tile([128, 2 * 4 * RU], BF16, tag="c_sb")
            nc.sync.dma_start(out=c_sb[:], in_=C[:, :])

            state = {}  # macro index -> tiles needed by the back-end

            def front(mi):
                gi, k = divmod(mi, GRP)
                if k == 0:
                    xg_t = xpool.tile(
                        [TILE_B, GRP, CHUNK, F], F32, tag="x", name="xt"
                    )
                    nc.sync.dma_start(out=xg_t[:], in_=X[gi])
                    state["xg"] = xg_t
                xm = state["xg"][:, k]  # [128, CHUNK, F]

                # ---- S = 1/sqrt(prod_f (1 + x^2)) ----
                sq = spool.tile([TILE_B, CHUNK, F], F32, tag="sq")
                nc.scalar.activation(sq[:], xm, AF.Square)
                sp1 = spool.tile([TILE_B, CHUNK, F], F32, tag="sp1")
                nc.vector.tensor_scalar_add(sp1[:], sq[:], 1.0)
                pr = spool.tile([TILE_B, CHUNK], F32, tag="pr")
                nc.vector.tensor_reduce(pr[:], sp1[:], AX.X, OP.mult)
                rp = spool.tile([TILE_B, CHUNK], F32, tag="rp")
                nc.vector.reciprocal(rp[:], pr[:])
                s_t = spool.tile([TILE_B, CHUNK], F32, tag="s_t")
                nc.scalar.activation(s_t[:], rp[:], AF.Sqrt)

                # ---- monomial halves pq[128, cg, 8] ----
                pq = mpool.tile([TILE_B, CG, 8], F32, tag="pq")
                if mi < 3:
                    nc.vector.memset(pq[:, :, 0:8:4], 1.0)
                xg = xm.rearrange("p c (g j) -> p (c g) j", j=4)
                src = xg.rearrange("p cg (h s) -> p cg h s", h=2)
                dst = pq[:].rearrange("p cg (h s) -> p cg h s", h=2)[
                    :, :, :, 1:3
                ]
                nc.gpsimd.tensor_copy(dst, src)
                dstm = pq[:].rearrange("p cg (h s) -> p cg h s", h=2)[
                    :, :, :, 3:4
                ]
                nc.gpsimd.tensor_tensor(
                    dstm, src[:, :, :, 0:1], src[:, :, :, 1:2], OP.mult
                )

                # ---- outer product Q[b, cg, i, j] (bf16) ----
                q = qpool.tile([TILE_B, CG, 4, 4], BF16, tag="q")
                pab_b = (
                    pq[:, :, 0:4].unsqueeze(3).broadcast_to([TILE_B, CG, 4, 4])
                )
                pcd_b = (
                    pq[:, :, 4:8].unsqueeze(2).broadcast_to([TILE_B, CG, 4, 4])
                )
                nc.vector.tensor_tensor(
                    q[:, 0:16], pab_b[:, 0:16], pcd_b[:, 0:16], OP.mult
                )
                nc.gpsimd.tensor_tensor(
                    q[:, 16:32], pab_b[:, 16:32], pcd_b[:, 16:32], OP.mult
                )

                # ---- transpose: qt[m, c, b] = q[b, (c, m)] ----
                qt = tpool.tile([128, CHUNK, TILE_B], BF16, tag="qt")
                nc.sync.dma_start_transpose(
                    qt[:], q[:].rearrange("p cg i j -> p (cg i j)")
                )
                state[mi] = {"qt": qt, "s_t": s_t}

            def matmuls(mi):
                st = state[mi]
                qt = st["qt"]
                psA, psB = [], []
                for c in range(CHUNK):
                    pa = pspool.tile(
                        [128, 512], F32, tag=f"pa{c}", name=f"pa{c}"
                    )
                    pb = pspool.tile(
                        [128, 512], F32, tag=f"pb{c}", name=f"pb{c}"
                    )
                    nc.tensor.matmul(
                        pa[:, 0:320],
                        qt[0:64, c, :],
                        c_sb[0:64, 0:320],
                        start=True,
                        stop=True,
                    )
                    nc.tensor.matmul(
                        pb[:, 0:320],
                        qt[64:128, c, :],
                        c_sb[64:128, 320:640],
                        start=True,
                        stop=True,
                    )
                    psA.append(pa)
                    psB.append(pb)
                st["psA"], st["psB"] = psA, psB

            def back(mi):
                st = state.pop(mi)
                psA, psB, s_t = st["psA"], st["psB"], st["s_t"]
                gi, k = divmod(mi, GRP)

                l1 = lpool.tile([TILE_B, CHUNK, 4, RU], BF16, tag="l1")
                for c in range(CHUNK):
                    eng = nc.vector if c < 2 else nc.gpsimd
                    eng.tensor_tensor(
                        l1[:, c],
                        psA[c][:, 0:320].rearrange("p (g k) -> p g k", g=4),
                        psB[c][:, 0:320].rearrange("p (g k) -> p g k", g=4),
                        OP.mult,
                    )
                l2 = lpool.tile([TILE_B, CHUNK, 2, RU], BF16, tag="l2")
                nc.vector.tensor_tensor(
                    l2[:], l1[:, :, 0:2], l1[:, :, 2:4], OP.mult
                )
                l3 = lpool.tile([TILE_B, CHUNK, RU], BF16, tag="l3")
                nc.vector.tensor_tensor(
                    l3[:], l2[:, :, 0], l2[:, :, 1], OP.mult
                )

                # ---- sum over rank (ru = u*10 + r) ----
                of = opool.tile([TILE_B, CHUNK, U], F32, tag="of")
                nc.vector.tensor_reduce(
                    of[:],
                    l3[:].rearrange("p c (u r) -> p c u r", r=R),
                    AX.X,
                    OP.add,
                )
                # ---- apply S, into the grouped store tile ----
                if k == 0:
                    state["ost"] = opool.tile(
                        [TILE_B, GRP, CHUNK, U], F32, tag="os", name="ost"
                    )
                os_ = state["ost"]
                nc.vector.tensor_tensor(
                    os_[:, k],
                    of[:],
                    s_t[:].unsqueeze(2).broadcast_to([TILE_B, CHUNK, U]),
                    OP.mult,
                )
                if k == GRP - 1:
                    nc.sync.dma_start(out=out[gi], in_=os_[:])

            # software-pipelined emission, fronts two macros ahead:
            #   front(m+1), back(m-1), matmuls(m)
            front(0)
            front(1)
            matmuls(0)
            for mi in range(1, N_MACRO):
                if mi + 1 < N_MACRO:
                    front(mi + 1)
                back(mi - 1)
                matmuls(mi)
            back(N_MACRO - 1)
    nc.finalize()
    return nc


def _pack_weights(kernel: np.ndarray):
    import ml_dtypes

    K = kernel.astype(np.float64)  # [2, R, F, U]
    C = np.zeros((128, 2 * 4 * RU), np.float64)
    bits = [(0, 0), (1, 0), (0, 1), (1, 1)]
    for g in range(NG):
        half = g // 4
        for i, (ba, bb) in enumerate(bits):
            for j, (bc, bd) in enumerate(bits):
                m = g * 16 + i * 4 + j
                coef = (
                    K[ba, :, 4 * g, :]
                    * K[bb, :, 4 * g + 1, :]
                    * K[bc, :, 4 * g + 2, :]
                    * K[bd, :, 4 * g + 3, :]
                )  # [R, U]
                col0 = half * 320 + (g % 4) * RU
                # ru = u*10 + r
                C[m, col0 : col0 + RU] = coef.T.reshape(RU)
    return C.astype(ml_dtypes.bfloat16)


_NC_CACHE = {}


def kernel(X: np.ndarray, kernel: np.ndarray) -> np.ndarray:
    if "nc" not in _NC_CACHE:
        _NC_CACHE["nc"] = build_nc()
    nc = _NC_CACHE["nc"]
    C = _pack_weights(kernel)
    X = np.ascontiguousarray(X, dtype=np.float32)
    # row b of core = gi*2048 + k*512 + c*128 + p  ->  [gi, p, k, c, f]
    Xd = (
        X.reshape(N_CORES, N_GRP, GRP, CHUNK, TILE_B, F)
        .transpose(0, 1, 4, 2, 3, 5)
        .copy()
    )
    in_maps = []
    for c in range(N_CORES):
        in_maps.append({"X": Xd[c], "C": C})
    res = run_bass_kernel_spmd(nc, in_maps, core_ids=list(range(N_CORES)))
    outs = []
    for c in range(N_CORES):
        o = res.results[c]["out"]  # [N_GRP, TILE_B, GRP, CHUNK, U]
        outs.append(o.transpose(0, 2, 3, 1, 4).reshape(B_CORE, U))
    return np.concatenate(outs, axis=0).astype(np.float32)


if __name__ == "__main__":
    rng = np.random.default_rng(0)
    X = rng.standard_normal((B_FULL, F), dtype=np.float32)
    K = (rng.standard_normal((2, R, F, U)) * 0.24).astype(np.float32)
    y = kernel(X, K)
    print(y.shape, y.dtype, np.abs(y).max())


# revision 9
# speedup vs baseline: 2.5487x; 1.0480x over previous
"""Trainium2 Bass kernel for nn_CP_Based (CP-decomposition feature-product layer).

Math: out[b,u] = sum_r prod_f ( x0[b,f]*K[0,r,f,u] + x1[b,f]*K[1,r,f,u] )
  with x0 = 1/sqrt(1+X^2), x1 = X/sqrt(1+X^2).
Factor the normalization out of the f-product:
  out[b,u] = S[b] * sum_r prod_f ( K0[f,ru] + X[b,f]*K1[f,ru] ),
  S[b] = 1/sqrt(prod_f (1+X[b,f]^2)).
The 32-feature product is decomposed into 8 groups of 4 features. Each group's
product is a linear map from the 16 multilinear monomials of its 4 features:
  G[b, g, ru] = sum_m Q[b, g, m] * C[g, m, ru]
Layout: batch rows sit on the PARTITION axis of the matmul OUTPUT, so each
matmul is (stationary QT[m, b-chunk]) x (moving C-block[m, (g,ru)]):
  - Q [128b, (c,g,i,j)] built on DVE/GPSIMD from monomial halves, stored bf16
  - QT via one DMA-transpose instruction (no PE transpose, no PSUM evacuation)
  - 2 matmuls per 128-row chunk: groups 0-3 (K=64) and groups 4-7 (K=64),
    each out [128, 320] into its own PSUM bank, bf16 moving = 1 cycle/row
  - product over 8 groups = 3-level elementwise chain (bf16, DVE 2x mode),
    level 1 reads the two PSUM banks directly
  - sum over rank r: strided tensor_reduce (ru packed u-major: ru = u*10+r)
  - S computed via Act Square + DVE (+1, prod-reduce, reciprocal) + Act Sqrt
    (Square and Sqrt share one act table set -> no per-macro table reloads)
The emission is software-pipelined one stage deep: each iteration emits the
pre-matmul front-end of macro m, then the post-matmul chain of macro m-1,
then the matmuls of macro m, so no engine queue head-of-line blocks on the
PSUM->chain dependency. X loads and output stores are batched 4 macros per
DMA to keep HWDGE occupancy low.

Sharding: pure data-parallel over batch: 131072 rows -> 8 cores x 16384.
"""

import sys

import numpy as np

sys.path.insert(0, "/opt/trn_rl_repo")

import concourse.bacc as bacc  # noqa: E402
import concourse.mybir as mybir  # noqa: E402
from concourse.bass_utils import run_bass_kernel_spmd  # noqa: E402
from concourse.tile import TileContext  # noqa: E402

F32 = mybir.dt.float32
BF16 = mybir.dt.bfloat16
AF = mybir.ActivationFunctionType
OP = mybir.AluOpType
AX = mybir.AxisListType

B_FULL = 131072
N_CORES = 8
B_CORE = B_FULL // N_CORES  # 16384
F = 32
R, U = 10, 8
RU = R * U  # 80
NG = 8  # feature groups of 4
TILE_B = 128
CHUNK = 4  # 128-row chunks per macro tile
MACRO_B = TILE_B * CHUNK  # 512
N_MACRO = B_CORE // MACRO_B  # 32
CG = CHUNK * NG  # 32 (chunk, group) pairs
GRP = 4  # macros per X-load / out-store DMA
N_GRP = N_MACRO // GRP  # 8


def build_nc():
    nc = bacc.Bacc()
    X = nc.dram_tensor(
        "X", [N_GRP, TILE_B, GRP, CHUNK, F], F32, kind="ExternalInput"
    )
    # C rows: m = g*16 + i*4 + j; cols: g*80 + u*10 + r (within-half blocks)
    C = nc.dram_tensor("C", [128, 2 * 4 * RU], BF16, kind="ExternalInput")
    out = nc.dram_tensor(
        "out", [N_GRP, TILE_B, GRP, CHUNK, U], F32, kind="ExternalOutput"
    )

    with TileContext(nc) as tc:
        with (
            tc.tile_pool(name="const", bufs=1) as cpool,
            tc.tile_pool(name="xin", bufs=3) as xpool,
            tc.tile_pool(name="sno", bufs=5) as spool,
            tc.tile_pool(name="mono", bufs=5) as mpool,
            tc.tile_pool(name="qq", bufs=5) as qpool,
            tc.tile_pool(name="qt", bufs=5) as tpool,
            tc.tile_pool(name="chain", bufs=3) as lpool,
            tc.tile_pool(name="outp", bufs=3) as opool,
            tc.tile_pool(name="psum", bufs=1, space="PSUM") as pspool,
        ):
            c_sb = cpool.tile([128, 2 * 4 * RU], BF16, tag="c_sb")
            nc.sync.dma_start(out=c_sb[:], in_=C[:, :])

            state = {}  # macro index -> tiles needed by the back-end

            def front(mi):
                gi, k = divmod(mi, GRP)
                if k == 0:
                    xg_t = xpool.tile(
                        [TILE_B, GRP, CHUNK, F], F32, tag="x", name="xt"
                    )
                    nc.sync.dma_start(out=xg_t[:], in_=X[gi])
                    state["xg"] = xg_t
                xm = state["xg"][:, k]  # [128, CHUNK, F]

                # ---- S = 1/sqrt(prod_f (1 + x^2)) ----
                sq = spool.tile([TILE_B, CHUNK, F], F32, tag="sq")
                nc.scalar.activation(sq[:], xm, AF.Square)
                sp1 = spool.tile([TILE_B, CHUNK, F], F32, tag="sp1")
                nc.vector.tensor_scalar_add(sp1[:], sq[:], 1.0)
                pr = spool.tile([TILE_B, CHUNK], F32, tag="pr")
                nc.vector.tensor_reduce(pr[:], sp1[:], AX.X, OP.mult)
                rp = spool.tile([TILE_B, CHUNK], F32, tag="rp")
                nc.vector.reciprocal(rp[:], pr[:])
                s_t = spool.tile([TILE_B, CHUNK], F32, tag="s_t")
                nc.scalar.activation(s_t[:], rp[:], AF.Sqrt)

                # ---- monomial halves pq[128, cg, 8] ----
                pq = mpool.tile([TILE_B, CG, 8], F32, tag="pq")
                if mi < 5:
                    nc.vector.memset(pq[:, :, 0:8:4], 1.0)
                xg = xm.rearrange("p c (g j) -> p (c g) j", j=4)
                src = xg.rearrange("p cg (h s) -> p cg h s", h=2)
                dst = pq[:].rearrange("p cg (h s) -> p cg h s", h=2)[
                    :, :, :, 1:3
                ]
                nc.gpsimd.tensor_copy(dst, src)
                dstm = pq[:].rearrange("p cg (h s) -> p cg h s", h=2)[
                    :, :, :, 3:4
                ]
                nc.gpsimd.tensor_tensor(
                    dstm, src[:, :, :, 0:1], src[:, :, :, 1:2], OP.mult
                )

                # ---- outer product Q[b, cg, i, j] (bf16) ----
                q = qpool.tile([TILE_B, CG, 4, 4], BF16, tag="q")
                pab_b = (
                    pq[:, :, 0:4].unsqueeze(3).broadcast_to([TILE_B, CG, 4, 4])
                )
                pcd_b = (
                    pq[:, :, 4:8].unsqueeze(2).broadcast_to([TILE_B, CG, 4, 4])
                )
                nc.vector.tensor_tensor(
                    q[:, 0:16], pab_b[:, 0:16], pcd_b[:, 0:16], OP.mult
                )
                nc.gpsimd.tensor_tensor(
                    q[:, 16:32], pab_b[:, 16:32], pcd_b[:, 16:32], OP.mult
                )

                # ---- transpose: qt[m, c, b] = q[b, (c, m)] ----
                qt = tpool.tile([128, CHUNK, TILE_B], BF16, tag="qt")
                nc.sync.dma_start_transpose(
                    qt[:], q[:].rearrange("p cg i j -> p (cg i j)")
                )
                state[mi] = {"qt": qt, "s_t": s_t}

            def matmuls(mi):
                st = state[mi]
                qt = st["qt"]
                psA, psB = [], []
                for c in range(CHUNK):
                    pa = pspool.tile(
                        [128, 512], F32, tag=f"pa{c}", name=f"pa{c}"
                    )
                    pb = pspool.tile(
                        [128, 512], F32, tag=f"pb{c}", name=f"pb{c}"
                    )
                    nc.tensor.matmul(
                        pa[:, 0:320],
                        qt[0:64, c, :],
                        c_sb[0:64, 0:320],
                        start=True,
                        stop=True,
                    )
                    nc.tensor.matmul(
                        pb[:, 0:320],
                        qt[64:128, c, :],
                        c_sb[64:128, 320:640],
                        start=True,
                        stop=True,
                    )
                    psA.append(pa)
                    psB.append(pb)
                st["psA"], st["psB"] = psA, psB

            def back(mi):
                st = state.pop(mi)
                psA, psB, s_t = st["psA"], st["psB"], st["s_t"]
                gi, k = divmod(mi, GRP)

                l1 = lpool.tile([TILE_B, CHUNK, 4, RU], BF16, tag="l1")
                for c in range(CHUNK):
                    eng = nc.vector if c < 2 else nc.gpsimd
                    eng.tensor_tensor(
                        l1[:, c],
                        psA[c][:, 0:320].rearrange("p (g k) -> p g k", g=4),
                        psB[c][:, 0:320].rearrange("p (g k) -> p g k", g=4),
                        OP.mult,
                    )
                l2 = lpool.tile([TILE_B, CHUNK, 2, RU], BF16, tag="l2")
                nc.vector.tensor_tensor(
                    l2[:], l1[:, :, 0:2], l1[:, :, 2:4], OP.mult
                )
                l3 = lpool.tile([TILE_B, CHUNK, RU], BF16, tag="l3")
                nc.vector.tensor_tensor(
                    l3[:], l2[:, :, 0], l2[:, :, 1], OP.mult
                )

                # ---- sum over rank (ru = u*10 + r) ----
                of = opool.tile([TILE_B, CHUNK, U], F32, tag="of")
                nc.vector.tensor_reduce(
                    of[:],
                    l3[:].rearrange("p c (u r) -> p c u r", r=R),
                    AX.X,
                    OP.add,
                )
                # ---- apply S, into the grouped store tile ----
                if k == 0:
                    state["ost"] = opool.tile(
                        [TILE_B, GRP, CHUNK, U], F32, tag="os", name="ost"
                    )
                os_ = state["ost"]
                nc.vector.tensor_tensor(
                    os_[:, k],
                    of[:],
                    s_t[:].unsqueeze(2).broadcast_to([TILE_B, CHUNK, U]),
                    OP.mult,
                )
                if k == GRP - 1:
                    nc.sync.dma_start(out=out[gi], in_=os_[:])

            # software-pipelined emission, fronts two macros ahead:
            #   front(m+1), back(m-1), matmuls(m)
            front(0)
            front(1)
            matmuls(0)
            for mi in range(1, N_MACRO):
                if mi + 1 < N_MACRO:
                    front(mi + 1)
                back(mi - 1)
                matmuls(mi)
            back(N_MACRO - 1)
    nc.finalize()
    return nc


def _pack_weights(kernel: np.ndarray):
    import ml_dtypes

    K = kernel.astype(np.float64)  # [2, R, F, U]
    C = np.zeros((128, 2 * 4 * RU), np.float64)
    bits = [(0, 0), (1, 0), (0, 1), (1, 1)]
    for g in range(NG):
        half = g // 4
        for i, (ba, bb) in enumerate(bits):
            for j, (bc, bd) in enumerate(bits):
                m = g * 16 + i * 4 + j
                coef = (
                    K[ba, :, 4 * g, :]
                    * K[bb, :, 4 * g + 1, :]
                    * K[bc, :, 4 * g + 2, :]
                    * K[bd, :, 4 * g + 3, :]
                )  # [R, U]
                col0 = half * 320 + (g % 4) * RU
                # ru = u*10 + r
                C[m, col0 : col0 + RU] = coef.T.reshape(RU)
    return C.astype(ml_dtypes.bfloat16)


_NC_CACHE = {}


def kernel(X: np.ndarray, kernel: np.ndarray) -> np.ndarray:
    if "nc" not in _NC_CACHE:
        _NC_CACHE["nc"] = build_nc()
    nc = _NC_CACHE["nc"]
    C = _pack_weights(kernel)
    X = np.ascontiguousarray(X, dtype=np.float32)
    # row b of core = gi*2048 + k*512 + c*128 + p  ->  [gi, p, k, c, f]
    Xd = (
        X.reshape(N_CORES, N_GRP, GRP, CHUNK, TILE_B, F)
        .transpose(0, 1, 4, 2, 3, 5)
        .copy()
    )
    in_maps = []
    for c in range(N_CORES):
        in_maps.append({"X": Xd[c], "C": C})
    res = run_bass_kernel_spmd(nc, in_maps, core_ids=list(range(N_CORES)))
    outs = []
    for c in range(N_CORES):
        o = res.results[c]["out"]  # [N_GRP, TILE_B, GRP, CHUNK, U]
        outs.append(o.transpose(0, 2, 3, 1, 4).reshape(B_CORE, U))
    return np.concatenate(outs, axis=0).astype(np.float32)


if __name__ == "__main__":
    rng = np.random.default_rng(0)
    X = rng.standard_normal((B_FULL, F), dtype=np.float32)
    K = (rng.standard_normal((2, R, F, U)) * 0.24).astype(np.float32)
    y = kernel(X, K)
    print(y.shape, y.dtype, np.abs(y).max())
